# revision 38
# baseline (speedup 1.0000x reference)
"""Trainium2 Bass kernel for nn_ConvolutionRefinement (final, 68.6 us/core).

Computes: silu(depthwise_causal_conv1d(rmsnorm(v) * norm_w) + bias) + v
over v_gated [B=4, H=16, L=4096, D=128], data-parallel over B*H across 8
cores (8 samples of [D, L] per core). Baseline: 142 us; this kernel: ~68.6 us.

Phase-interleaved layout: the host stages each sample as fp16 [128, 4096]
with partition p = 4c + r holding channel c + 32g, column g*1024 + j holding
time t = 4j + r (4 channel groups g side by side). In this layout:

  - The K=4 causal depthwise conv runs as 2 matmuls per group ("aligned" +
    "carry") with 128x128 block stationaries: each moving column carries 4
    time phases x 32 channels, so each output column finishes 2 taps per
    channel -> 2 useful MACs/PE-row/cycle, i.e. half the PE time of the
    naive per-tap diagonal approach. The carry matmul skips output column 0
    (causal zero), eliminating pad columns entirely.
  - sum_d x^2 per position: ones-block-stationary matmuls accumulated over
    the 4 groups; the stationary replicates the result to all 128
    partitions for free, making the rsqrt chain and x*inv plain elementwise.
  - rsqrt(mean x^2): ACT Square seed q = (AL*s + BE)^2 (Square lives in
    Silu's activation table -> zero table reloads) + one fused Newton-like
    step inv = q*(RA + RB*s*q^2) on DVE (constants fitted offline to 4.7e-3
    ripple; output scale folded into the conv stationaries via WFOLD).

Engine budget per sample (cost model): DVE 6.7us (sq g1-3, chain, xh g0-2,
resid g0), ACT 6.2us (sq g0, seed, 4x silu), PE 5.3us (24 matmuls), Pool
5.0us (xh g3, resid g1-3, SWDGE stores). DMA issue is spread so per-engine
DMA lanes overlap: loads on SP, consts on ACT, stores on Pool, with the
pipeline tail's stores fanned across lanes. Emission is software-pipelined
(load / mid / conv / back stages with skew) and the PE is pre-warmed with
dummy matmuls to reach full p-state before the first real matmul.
All on-chip data is fp16 (2-byte DVE fast modes, ~8x less rounding noise
than bf16): rel err ~3.7e-3 vs the fp32 reference.
"""

import sys

if "/opt/trn_rl_repo" not in sys.path:
    sys.path.insert(0, "/opt/trn_rl_repo")

import numpy as np

B, H, L, D, K = 4, 16, 4096, 128, 4
NCORES = 8
S = (B * H) // NCORES  # samples per core
J = L // 4             # columns per group (phase-interleaved)
GW = J + 1             # group width in the padded x/xh tiles

# rsqrt composite constants (fitted offline, ripple 4.7e-3; fp16 chain 5.8e-3)
AL = -0.002059129248087416
BE = 1.3306419197548047
RA = 1.7910646266976955
RB = -0.004181191851042138
LAM = 14.201079344975795
WFOLD = float(np.sqrt(128.0) / LAM)

_CACHE = {}


CFG = {
    "xs_bufs": 5, "sq_bufs": 2, "st_bufs": 2, "xh_bufs": 2,
    "silu_bufs": 2, "out_bufs": 2, "ss_bufs": 2, "cv_bufs": 2,
    "xh_dve_g": 3,     # groups 0..xh_dve_g-1 on DVE, rest Pool
    "resid_dve_g": 1,  # groups 0..resid_dve_g-1 on DVE, rest Pool
    "sq_act": True,    # group-0 square on ACT
    "split_tail_stores": True,
    "tail_resid_dve_g": 4,
    "pe_warmup": 8,
    "tail_n": 3,
    "split_load0": True,
    "sq_pool": False,  # group-0 square on Pool instead (overrides sq_act)
    "act_loads": False, # odd loads issued from ACT
}


def _build_nc(cfg=None):
    cfg = dict(CFG, **(cfg or {}))
    import concourse.bass as bass
    import concourse.mybir as mybir
    from concourse.tile import TileContext

    fp32 = mybir.dt.float32
    fp16 = mybir.dt.float16
    Alu = mybir.AluOpType
    Act = mybir.ActivationFunctionType

    import bass_rust

    def _split_sync_waits(nc):
        ctr = 0
        for f in nc.m.functions:
            for blk in f.blocks:
                new = []
                for inst in blk.instructions:
                    si = inst.sync_info
                    waits = list(si.on_wait) if si and si.on_wait else []
                    if len(waits) > 1:
                        for w in waits[:-1]:
                            nop = mybir.InstNoOp(
                                name=f"wsplit-{ctr}", ins=[], outs=[]
                            )
                            ctr += 1
                            nop.engine = inst.engine
                            nop.sync_info = bass_rust.SyncInfo(
                                on_wait=[w], on_update=[]
                            )
                            nc.register_instruction(nop)
                            new.append(nop)
                        inst.sync_info = bass_rust.SyncInfo(
                            on_wait=[waits[-1]],
                            on_update=list(si.on_update or []),
                        )
                    new.append(inst)
                blk.instructions = new

    nc = bass.Bass(trn_type="TRN2")
    x_dram = nc.dram_tensor("x", [S, 128, L], fp16, kind="ExternalInput")
    cst_dram = nc.dram_tensor("cst", [128, 9 * 128], fp16, kind="ExternalInput")
    bias_dram = nc.dram_tensor("bias", [128, 4], fp32, kind="ExternalInput")
    y_dram = nc.dram_tensor("y", [S, 128, L], fp16, kind="ExternalOutput")

    with TileContext(nc) as tc:
        with (
            tc.tile_pool(name="const", bufs=1) as constp,
            tc.tile_pool(name="xs", bufs=cfg["xs_bufs"]) as xp,
            tc.tile_pool(name="sq", bufs=cfg["sq_bufs"]) as sqp,
            tc.tile_pool(name="st", bufs=cfg["st_bufs"]) as stp,
            tc.tile_pool(name="xh", bufs=cfg["xh_bufs"]) as xhp,
            tc.tile_pool(name="silu", bufs=cfg["silu_bufs"]) as slp,
            tc.tile_pool(name="out", bufs=cfg["out_bufs"]) as outp,
            tc.tile_pool(name="ss_ps", bufs=cfg["ss_bufs"], space="PSUM") as ssp,
            tc.tile_pool(name="cv_ps", bufs=cfg["cv_bufs"], space="PSUM") as cvp,
        ):
            cst_sb = constp.tile([128, 9 * 128], fp16)
            nc.scalar.dma_start(out=cst_sb[:], in_=cst_dram[:])
            sa_sb = cst_sb[:, 0 : 4 * 128]
            sb_sb = cst_sb[:, 4 * 128 : 8 * 128]
            so_sb = cst_sb[:, 8 * 128 : 9 * 128]
            b_sb = constp.tile([128, 4], fp32)
            nc.scalar.dma_start(out=b_sb[:], in_=bias_dram[:])
            be_sb = constp.tile([128, 1], fp32)
            nc.vector.memset(be_sb[:], BE)
            z_sb = constp.tile([128, 1], fp32)
            nc.vector.memset(z_sb[:], 0.0)

            nwarm = cfg.get("pe_warmup", 0)
            if nwarm:
                wps = cvp.tile([128, J], fp32, tag="cv")
                for i in range(nwarm):
                    nc.tensor.matmul(
                        wps[:, 0:512], so_sb, cst_sb[:, 0:512],
                        start=True, stop=True,
                    )

            xts = [None] * S
            xhs = [None] * S
            silus = [None] * S

            def emit_load(s):
                x_t = xp.tile([128, L], fp16, tag="x")
                if cfg.get("split_load0", False) and s == 0:
                    if cfg.get("load0_3way", False):
                        nc.sync.dma_start(out=x_t[:, 0:J], in_=x_dram[s, :, 0:J])
                        nc.gpsimd.dma_start(out=x_t[:, J : 2 * J],
                                            in_=x_dram[s, :, J : 2 * J])
                        nc.scalar.dma_start(out=x_t[:, 2 * J :],
                                            in_=x_dram[s, :, 2 * J :])
                    else:
                        nc.sync.dma_start(out=x_t[:, 0 : L // 2],
                                          in_=x_dram[s, :, 0 : L // 2])
                        nc.gpsimd.dma_start(out=x_t[:, L // 2 :],
                                            in_=x_dram[s, :, L // 2 :])
                elif cfg.get("fan_first_loads", False) and s in (1, 2):
                    eng = nc.scalar if s == 1 else nc.gpsimd
                    eng.dma_start(out=x_t[:], in_=x_dram[s])
                elif cfg.get("split_loads", False):
                    nc.sync.dma_start(out=x_t[:, 0 : L // 2],
                                      in_=x_dram[s, :, 0 : L // 2])
                    nc.scalar.dma_start(out=x_t[:, L // 2 :],
                                        in_=x_dram[s, :, L // 2 :])
                elif s % 2 == 0 or not cfg["act_loads"]:
                    nc.sync.dma_start(out=x_t[:], in_=x_dram[s])
                else:
                    nc.scalar.dma_start(out=x_t[:], in_=x_dram[s])
                xts[s] = x_t

            def emit_mid(s):
                x_t = xts[s]
                # sq = x*x: group 0 on ACT (Square), groups 1-3 on DVE
                sq_t = sqp.tile([128, L], fp16, tag="sq")
                if cfg["sq_pool"]:
                    nc.gpsimd.tensor_tensor(
                        sq_t[:, 0:J], x_t[:, 0:J], x_t[:, 0:J], Alu.mult
                    )
                    nc.vector.tensor_tensor(
                        sq_t[:, J:], x_t[:, J:], x_t[:, J:], Alu.mult
                    )
                elif cfg["sq_act"]:
                    na = cfg.get("sq_act_g", 1)
                    for g in range(na):
                        nc.scalar.activation(sq_t[:, g * J : (g + 1) * J],
                                             x_t[:, g * J : (g + 1) * J],
                                             Act.Square,
                                             bias=z_sb[:, 0:1], scale=1.0)
                    if na > 1:
                        nc.vector.tensor_tensor(
                            sq_t[:, na * J :], x_t[:, na * J :],
                            x_t[:, na * J :], Alu.mult
                        )
                    elif cfg.get("sq_split", False):
                        for g in range(1, 4):
                            nc.vector.tensor_tensor(
                                sq_t[:, g * J : (g + 1) * J],
                                x_t[:, g * J : (g + 1) * J],
                                x_t[:, g * J : (g + 1) * J], Alu.mult
                            )
                    else:
                        nc.vector.tensor_tensor(
                            sq_t[:, J:], x_t[:, J:], x_t[:, J:], Alu.mult
                        )
                else:
                    nc.vector.tensor_tensor(
                        sq_t[:], x_t[:], x_t[:], Alu.mult
                    )
                # sumsq over channels (replicated)
                ss = ssp.tile([128, J], fp32, tag="ss")
                for g in range(4):
                    for h in range(2):
                        nc.tensor.matmul(
                            ss[:, 512 * h : 512 * (h + 1)],
                            so_sb,
                            sq_t[:, g * J + 512 * h : g * J + 512 * (h + 1)],
                            start=(g == 0),
                            stop=(g == 3),
                        )
                # rsqrt chain
                q0 = stp.tile([128, J], fp16, tag="q0")
                w_t = stp.tile([128, J], fp16, tag="w")
                u_t = stp.tile([128, J], fp16, tag="u")
                v_t = stp.tile([128, J], fp16, tag="v")
                inv = stp.tile([128, J], fp16, tag="inv")
                nh = 2 if (cfg.get("chain_halves", False) or
                           (s < cfg.get("chain_halves_head", 0))) else 1
                cw = J // nh
                weng = nc.gpsimd if cfg.get("w_pool", False) else nc.vector
                w_act = cfg.get("w_act", False)
                veng = nc.gpsimd if cfg.get("v_pool", False) else nc.vector
                for hh in range(nh):
                    sl = slice(cw * hh, cw * (hh + 1))
                    nc.scalar.activation(q0[:, sl], ss[:, sl], Act.Square,
                                         bias=be_sb[:, 0:1], scale=AL)
                    if w_act:
                        nc.scalar.activation(w_t[:, sl], q0[:, sl], Act.Square,
                                             bias=z_sb[:, 0:1], scale=1.0)
                    else:
                        weng.tensor_tensor(w_t[:, sl], q0[:, sl], q0[:, sl], Alu.mult)
                    if cfg.get("ss16_pool", False):
                        s16 = stp.tile([128, J], fp16, tag="s16")
                        nc.gpsimd.tensor_scalar(s16[:, sl], ss[:, sl], RB, None, Alu.mult)
                        nc.vector.tensor_tensor(
                            u_t[:, sl], s16[:, sl], w_t[:, sl], Alu.mult
                        )
                    else:
                        nc.vector.scalar_tensor_tensor(
                            u_t[:, sl], ss[:, sl], RB, w_t[:, sl], Alu.mult, Alu.mult
                        )
                    veng.tensor_scalar(v_t[:, sl], u_t[:, sl], RA, None, Alu.add)
                    nc.vector.tensor_tensor(inv[:, sl], v_t[:, sl], q0[:, sl], Alu.mult)
                # xh = x*inv
                xh = xhp.tile([128, L], fp16, tag="xh")
                nd = cfg["xh_dve_g"]
                for g in range(nd):
                    nc.vector.tensor_tensor(
                        xh[:, g * J : (g + 1) * J],
                        x_t[:, g * J : (g + 1) * J],
                        inv[:],
                        Alu.mult,
                    )
                for g in range(nd, 4):
                    nc.gpsimd.tensor_tensor(
                        xh[:, g * J : (g + 1) * J],
                        x_t[:, g * J : (g + 1) * J],
                        inv[:],
                        Alu.mult,
                    )
                xhs[s] = xh

            def emit_conv(s):
                xh = xhs[s]
                silu_sb = slp.tile([128, L], fp16, tag="silu")
                for g in range(4):
                    cv = cvp.tile([128, J], fp32, tag="cv")
                    for h in range(2):
                        nc.tensor.matmul(
                            cv[:, 512 * h : 512 * (h + 1)],
                            cst_sb[:, g * 128 : (g + 1) * 128],
                            xh[:, g * J + 512 * h : g * J + 512 * (h + 1)],
                            start=True,
                            stop=False,
                        )
                    # carry taps: out col 0 gets zero carry (causal), so the
                    # B matmuls cover out cols [1, 1024) reading cols [0, 1023)
                    nc.tensor.matmul(
                        cv[:, 1:512],
                        cst_sb[:, 512 + g * 128 : 512 + (g + 1) * 128],
                        xh[:, g * J : g * J + 511],
                        start=False,
                        stop=True,
                    )
                    nc.tensor.matmul(
                        cv[:, 512:1024],
                        cst_sb[:, 512 + g * 128 : 512 + (g + 1) * 128],
                        xh[:, g * J + 511 : g * J + 1023],
                        start=False,
                        stop=True,
                    )
                    if s == S - 1 and cfg.get("tail_silu_fine", False):
                        for h in range(2):
                            nc.scalar.activation(
                                silu_sb[:, g * J + 512 * h : g * J + 512 * (h + 1)],
                                cv[:, 512 * h : 512 * (h + 1)],
                                Act.Silu,
                                bias=b_sb[:, g : g + 1],
                                scale=1.0,
                            )
                    else:
                        nc.scalar.activation(
                            silu_sb[:, g * J : (g + 1) * J],
                            cv[:],
                            Act.Silu,
                            bias=b_sb[:, g : g + 1],
                            scale=1.0,
                        )
                silus[s] = silu_sb

            def emit_back(s):
                x_t, silu_sb = xts[s], silus[s]
                out_sb = outp.tile([128, L], fp16, tag="out")
                rg = cfg["resid_dve_g"]
                if s >= S - 1 and "tail_resid_dve_g" in cfg:
                    rg = cfg["tail_resid_dve_g"]
                if s == S - 1 and cfg.get("tail_fine", True):
                    engs = [nc.gpsimd, nc.sync] * 4
                    for h8 in range(8):
                        sl = slice(512 * h8, 512 * (h8 + 1))
                        nc.vector.tensor_tensor(
                            out_sb[:, sl], silu_sb[:, sl], x_t[:, sl], Alu.add
                        )
                        engs[h8].dma_start(out=y_dram[s, :, sl], in_=out_sb[:, sl])
                    xts[s] = None
                    silus[s] = None
                    return
                for g in range(4):
                    eng = nc.vector if g < rg else nc.gpsimd
                    eng.tensor_tensor(
                        out_sb[:, g * J : (g + 1) * J],
                        silu_sb[:, g * J : (g + 1) * J],
                        x_t[:, g * J : (g + 1) * J],
                        Alu.add,
                    )
                if cfg.get("tail_quarters", True) and s == S - 1:
                    engs = [nc.gpsimd, nc.sync, nc.gpsimd, nc.sync]
                    for g in range(4):
                        engs[g].dma_start(out=y_dram[s, :, g * J : (g + 1) * J],
                                          in_=out_sb[:, g * J : (g + 1) * J])
                elif cfg.get("split_all_stores", False):
                    nc.gpsimd.dma_start(out=y_dram[s, :, 0 : L // 2],
                                        in_=out_sb[:, 0 : L // 2])
                    nc.sync.dma_start(out=y_dram[s, :, L // 2 :],
                                      in_=out_sb[:, L // 2 :])
                elif cfg.get("split_tail_stores", False) and s >= S - cfg.get("tail_n", 2):
                    if cfg.get("tail3", False):
                        t3 = L // 3 // 512 * 512
                        nc.gpsimd.dma_start(out=y_dram[s, :, 0:t3],
                                            in_=out_sb[:, 0:t3])
                        nc.sync.dma_start(out=y_dram[s, :, t3 : 2 * t3],
                                          in_=out_sb[:, t3 : 2 * t3])
                        nc.scalar.dma_start(out=y_dram[s, :, 2 * t3 :],
                                            in_=out_sb[:, 2 * t3 :])
                    else:
                        nc.gpsimd.dma_start(out=y_dram[s, :, 0 : L // 2],
                                            in_=out_sb[:, 0 : L // 2])
                        nc.sync.dma_start(out=y_dram[s, :, L // 2 :],
                                          in_=out_sb[:, L // 2 :])
                else:
                    nc.gpsimd.dma_start(out=y_dram[s], in_=out_sb[:])

            skew_conv = cfg.get("skew_conv", 1)
            skew_back = cfg.get("skew_back", 2)
            for it in range(S + skew_back + 1):
                if it < S:
                    emit_load(it)
                m = it - 1
                if 0 <= m < S:
                    emit_mid(m)
                c = it - 1 - skew_conv
                if 0 <= c < S:
                    emit_conv(c)
                b = it - 1 - skew_back
                if 0 <= b < S:
                    emit_back(b)

    _split_sync_waits(nc)
    return nc


def _get_nc():
    if "nc" not in _CACHE:
        _CACHE["nc"] = _build_nc()
    return _CACHE["nc"]


def _host_consts(norm_weight, conv_weight, conv_bias):
    nw = np.asarray(norm_weight, dtype=np.float64)
    cw = np.asarray(conv_weight, dtype=np.float64)
    w2 = cw * nw[:, None] * WFOLD  # [D, K] folded weights

    sa = np.zeros((4, 128, 128), dtype=np.float32)  # aligned (moving col j)
    sb = np.zeros((4, 128, 128), dtype=np.float32)  # carry (moving col j-1)
    for g in range(4):
        for c in range(32):
            d = c + 32 * g
            for rp in range(4):
                for k in range(K):
                    q = rp + k - 3
                    if q >= 0:
                        sa[g, 4 * c + q, 4 * c + rp] = w2[d, k]
                    else:
                        sb[g, 4 * c + q + 4, 4 * c + rp] = w2[d, k]
    so = np.zeros((128, 128), dtype=np.float32)
    for c in range(32):
        for r in range(4):
            for cp in range(32):
                so[4 * c + r, 4 * cp + r] = 1.0
    bias = np.zeros((128, 4), dtype=np.float32)
    for g in range(4):
        for c in range(32):
            for r in range(4):
                bias[4 * c + r, g] = conv_bias[c + 32 * g]
    blob = np.zeros((128, 9 * 128), dtype=np.float16)
    for g in range(4):
        blob[:, g * 128 : (g + 1) * 128] = sa[g].astype(np.float16)
        blob[:, 512 + g * 128 : 512 + (g + 1) * 128] = sb[g].astype(np.float16)
    blob[:, 1024:1152] = so.astype(np.float16)
    return blob, bias.astype(np.float32)


def _host_stage_input(v_gated):
    # [B,H,L,D] fp32 -> per-sample interleaved padded [BH, 128, 4*GW] fp16
    v = np.asarray(v_gated, dtype=np.float32).reshape(B * H, L, D)
    x = v.transpose(0, 2, 1)                    # [BH, D, L]
    xr = x.reshape(B * H, D, J, 4)              # [BH, d, j, r]
    xr = xr.transpose(0, 1, 3, 2)               # [BH, d, r, j]
    xg = xr.reshape(B * H, 4, 32, 4, J)         # [BH, g, c, r, j]
    xp = xg.reshape(B * H, 4, 128, J)           # [BH, g, p=4c+r, j]
    return np.ascontiguousarray(xp.reshape(B * H, 4, 128, J).transpose(0, 2, 1, 3)
                                .reshape(B * H, 128, L)).astype(np.float16)


def _host_unstage_output(y):
    # y: [BH, 128, L] fp16 with col g*J+j, partition 4c+r = out[c+32g, 4j+r]
    yr = np.asarray(y, dtype=np.float32).reshape(B * H, 32, 4, 4, J)
    # axes: (bh, c, r, g, j); out[bh, c+32g, 4j+r] = yr[bh, c, r, g, j]
    out = np.zeros((B * H, D, L), dtype=np.float32)
    for g in range(4):
        for r in range(4):
            out[:, 32 * g : 32 * (g + 1), r::4] = yr[:, :, r, g, :]
    return out


def kernel(v_gated, norm_weight, conv_weight, conv_bias):
    from concourse.bass_utils import run_bass_kernel_spmd

    nc = _get_nc()
    xt = _host_stage_input(v_gated)
    blob, bias = _host_consts(norm_weight, conv_weight, conv_bias)

    in_maps = []
    for c in range(NCORES):
        in_maps.append(
            {
                "x": np.ascontiguousarray(xt[c * S : (c + 1) * S]),
                "cst": blob,
                "bias": bias,
            }
        )
    res = run_bass_kernel_spmd(nc, in_maps, core_ids=list(range(NCORES)))
    y = np.concatenate(
        [np.asarray(r["y"], dtype=np.float32) for r in res.results], axis=0
    )
    out = _host_unstage_output(y)  # [BH, D, L]
    return out.transpose(0, 2, 1).reshape(B, H, L, D).astype(np.float32)
